# revision 24
# baseline (speedup 1.0000x reference)
"""Trainium2 Bass kernel for nn_CustomParameterTransform (scatter_memory).

Reference semantics: coord_v [256, 30] holds 10 (x, y, mass) triplets per
sample. Each triplet maps to integer grid indices (x_i, y_i, m_i); a one-hot
volume z [B, 16, 128, 128] is scattered (z[b, m, y, x] = 1) and the output is
concat(1-z, z) over the channel axis -> [256, 32, 128, 128] f32 (512 MB).

Strategy (8 NeuronCores, batch-sharded, no cross-core comm):
  - The output is almost entirely constant (first 16 channels 1.0, last 16
    0.0, except at 640 scatter points per core). Per core: one 64 MB
    write-only region built from SBUF "slab image" tiles whose
    partition-major sweep reproduces whole slabs (alternating 1 MB ones /
    1 MB zeros), so every fill is a contiguous DRAM write and both DMA
    sides stay 2-D (the HWDGE PDMA2D fast path; 3-D/strided APs demote to
    an engine-sequenced slow path measured ~5x slower).
  - Steady-state throughput is capped by the per-core DMA port (~435
    GB/s); a ring dispatches ~4 descriptors/us per outstanding
    instruction, so the ramp is limited by how quickly fill instructions
    become ready. Hence: a [128, 1024] mini tile (one ~0.9 us memset per
    engine) feeds the very first fills, slabs 1-5 are ten 1 MB half-slab
    fills (lots of outstanding instructions early), and slabs 6+ are 4 MB
    fills from a [128, 8192] tile (32 KB rows; rows can't exceed 32 KB
    because a larger slab image needs its value to alternate every <32
    partitions and compute APs must start on 32-partition quadrant
    boundaries).
  - A gpsimd software-DGE fill queue was tried as a third descriptor
    stream and made things worse: engines stall fetching software
    descriptors, throttling the HWDGE rings. Everything stays on the two
    rings.
  - The 640 scatter points are fixed up with indirect (scatter) DMAs whose
    deps are wired to just the fills covering their samples, so all but
    the last column overlap the fill phase.
  - The stock const-AP all-engine barrier in Bass.__init__ is patched out
    (nothing here uses const_aps) and TileContext's epilogue is replaced
    with a light drain, since the event-lowered sem-clear cascade scales
    with instruction count.
  - Indices are computed on the host with the exact same jax ops as the
    reference (bit-identical floor/log10 behavior) and passed per-core as
    a [128, 5] int32 tensor of flat element offsets.
"""

import numpy as np

B = 256
NSRC = 10
NMC = 16
L = 128
NCORES = 8
BL = B // NCORES          # 32 samples per core
PLANE = L * L             # 16384
HALF = NMC * PLANE        # 262144 elements per half-slab
SLAB = 2 * HALF           # 524288 elements per sample
OUT_ELEMS = BL * SLAB     # 16777216 per core (64 MB)

N_SCATTER_COLS = 5        # 640 scatter writes = 128 partitions x 5 columns
PTS = BL * NSRC           # 320 points per core

_CACHE = {}


def _build_nc():
    import concourse.bass as bass
    import concourse.tile as tile
    from concourse import bacc, mybir
    from concourse.tile_rust import add_dep_helper

    import types as _types
    from concourse.vector_clock import ScopedClock

    # The const-AP registration in Bass.__init__ ends with an all-engine
    # barrier (~1.5 us of event-sem chaining at the head of every
    # execution). This kernel never touches const_aps -- memset packs its
    # immediate and the DMAs don't use them -- so elide the barrier for
    # the duration of construction.
    _orig_barrier = bass.Bass.all_engine_barrier
    bass.Bass.all_engine_barrier = lambda self, **kw: None
    try:
        nc = bacc.Bacc("TRN2", target_bir_lowering=False, debug=False,
                       num_devices=NCORES)
    finally:
        bass.Bass.all_engine_barrier = _orig_barrier

    def _light_drain_and_barrier(self, tick_clock, wait_clock):
        """Replaces TileContext._drain_and_barrier for this kernel. The
        stock epilogue is drain + two all-engine EVSEM butterfly barriers
        around the sem clear (~9 us after event lowering). Requirements at
        kernel end are: (1) all DMA completions observed, (2) sems cleared
        for NEFF re-execution, (3) the clear happens after every engine's
        last sem use. (1) is the sync drain's global-clock waits; (3) is a
        counting-sem join (sync arrives only after the drain, so join>=4
        implies all DMA done); (2) is the ranged clear. The second barrier
        is unnecessary: a re-execution cannot start until every engine --
        including the clearing gpsimd -- has ended."""
        nc_ = self.nc
        drain_inst = nc_.sync.drain()
        wait_clock.add_sem_waits(
            drain_inst.ins, ScopedClock({None: tick_clock.global_clock}))
        join = nc_.alloc_semaphore("tail_join")
        for eng in nc_.engines.values():
            if eng is not nc_.gpsimd:
                eng.sem_inc(join, 1)
        n_other = len(nc_.engines) - 1
        nc_.gpsimd.wait_ge(join, n_other)
        popped = nc_._tile_sem_poison_stack.pop()
        assert popped is self._sem_poison
        sems = list(self.sems.allocated().values())
        nc_.clear_and_free_semaphores(sems + [join])

    offs = nc.dram_tensor("offs", [128, N_SCATTER_COLS], mybir.dt.int32,
                          kind="ExternalInput").ap()
    out = nc.dram_tensor("out", [BL, SLAB], mybir.dt.float32,
                         kind="ExternalOutput").ap()

    with tile.TileContext(nc) as tc:
        tc._drain_and_barrier = _types.MethodType(_light_drain_and_barrier, tc)
        with tc.tile_pool(name="src", bufs=1) as src_pool, \
             tc.tile_pool(name="small", bufs=1) as small_pool:
            # Mini tiles: first memset on each of vector/gpsimd (~0.9 us)
            # so the rings' first fills push as early as possible.
            ones_mini = src_pool.tile([128, 1024], mybir.dt.float32)
            zeros_mini = src_pool.tile([128, 1024], mybir.dt.float32)
            nc.vector.memset(ones_mini[:, :], 1.0)
            nc.gpsimd.memset(zeros_mini[:, :], 0.0)
            # Stage A [128, 4096] (1 slab/sweep, 16 KB rows): ones rows
            # ready next (~2 us later), zeros rows after that.
            slab_a = src_pool.tile([128, 4096], mybir.dt.float32)
            nc.vector.memset(slab_a[0:64, 0:2048], 1.0)
            nc.gpsimd.memset(slab_a[0:64, 2048:4096], 1.0)
            nc.vector.memset(slab_a[64:128, 0:2048], 0.0)
            nc.gpsimd.memset(slab_a[64:128, 2048:4096], 0.0)
            # Stage B [128, 8192] (2 slabs/sweep, 32 KB rows, value
            # alternating every 32 rows), columns split vector/gpsimd
            # (scalar and sync cannot memset).
            slab_b = src_pool.tile([128, 8192], mybir.dt.float32)
            for r in range(4):
                v = 1.0 if r % 2 == 0 else 0.0
                nc.vector.memset(slab_b[r * 32:(r + 1) * 32, 0:4096], v)
                nc.gpsimd.memset(slab_b[r * 32:(r + 1) * 32, 4096:8192], v)

            # Scatter offsets: [128, 5] int32 flat element indices.
            # Column j: rows 0-63 = ones-half offsets of points
            # 64j..64j+63 (write 0.0), rows 64-127 = z-half offsets of the
            # same points (write 1.0) -- vals_t is just two quadrant-
            # aligned memsets. These queue behind the gpsimd memsets; the
            # scatters need them ~50 us in.
            offs_t = small_pool.tile([128, N_SCATTER_COLS], mybir.dt.int32)
            nc.gpsimd.dma_start(offs_t[:, :], offs[:, :])
            vals_t = small_pool.tile([128, N_SCATTER_COLS], mybir.dt.float32)
            nc.gpsimd.memset(vals_t[0:64, :], 0.0)
            nc.gpsimd.memset(vals_t[64:128, :], 1.0)

            # Fills. sample_fills[s] lists the fills that write slab s.
            #   slab 0:    4 half-MB fills from the minis (earliest start)
            #   slabs 1-5: ten 1 MB half-slab fills from stage A -- many
            #              small instructions so the rings have descriptor
            #              sources queued while stage B memsets finish
            #   slabs 6-27: eleven 4 MB stage-B fills on the rings
            #   slabs 28-31: two 4 MB stage-B fills on gpsimd's software
            #              DGE queue (third descriptor stream)
            sample_fills = {s: [] for s in range(BL)}
            for k in range(2):
                f = nc.sync.dma_start(
                    out[0:1, k * HALF // 2:(k + 1) * HALF // 2],
                    ones_mini[:, :])
                sample_fills[0].append(f)
                f = nc.scalar.dma_start(
                    out[0:1, HALF + k * HALF // 2:HALF + (k + 1) * HALF // 2],
                    zeros_mini[:, :])
                sample_fills[0].append(f)
            for s in range(1, 6):
                f = nc.sync.dma_start(out[s:s + 1, 0:HALF], slab_a[0:64, :])
                sample_fills[s].append(f)
                f = nc.scalar.dma_start(out[s:s + 1, HALF:SLAB],
                                        slab_a[64:128, :])
                sample_fills[s].append(f)
            # Slabs 6-13: 2 MB single-slab fills (more outstanding
            # instructions right as stage B comes ready -- the ring
            # dispatches ~4 descriptors/us per outstanding instruction).
            # Slabs 14-29: 4 MB fills. Slabs 30-31 split 1-each so both
            # rings carry exactly 32 MB (an imbalanced ring shows up as
            # ~7 us of late engines on the heavy ring).
            for i, s in enumerate(range(6, 14)):
                eng = nc.sync if i % 2 == 0 else nc.scalar
                f = eng.dma_start(out[s:s + 1, :], slab_b[0:64, :])
                sample_fills[s].append(f)
            for i, s in enumerate(range(14, 30, 2)):
                eng = nc.sync if i % 2 == 0 else nc.scalar
                f = eng.dma_start(out[s:s + 2, :].flatten(), slab_b[:, :])
                for ss in (s, s + 1):
                    sample_fills[ss].append(f)
            sample_fills[30].append(
                nc.sync.dma_start(out[30:31, :], slab_b[0:64, :]))
            sample_fills[31].append(
                nc.scalar.dma_start(out[31:32, :], slab_b[0:64, :]))

            # Scatter columns: col j covers points 64j..64j+63, i.e.
            # samples floor(6.4j)..floor(6.4j+6.4).
            def deps(lo, hi):
                seen = []
                for s in range(lo, hi + 1):
                    for f in sample_fills[s]:
                        if f not in seen:
                            seen.append(f)
                return seen
            col_deps = [deps(0, 6), deps(6, 12), deps(12, 19),
                        deps(19, 25), deps(25, 31)]

            # Narrow declared out AP ([1, 1] at offset 0): the real write
            # addresses come from the offset tensor; a full-tensor AP would
            # make Tile serialize every scatter behind every fill (WAW), and
            # the explicit col_deps edges below provide the true ordering.
            out2d = out[0:1, 0:1]
            for j in range(N_SCATTER_COLS):
                sc = nc.gpsimd.indirect_dma_start(
                    out=out2d,
                    out_offset=bass.IndirectOffsetOnAxis(
                        ap=offs_t[:, j:j + 1], axis=0),
                    in_=vals_t[:, j:j + 1],
                    in_offset=None,
                )
                for fl in col_deps[j]:
                    add_dep_helper(sc.ins, fl.ins,
                                   reason="scatter after its sample fills")

    nc.compile()
    return nc


def _compute_indices(coord_v, lows, highs, nmc, L_):
    """Replicates reference.py lines exactly (same jax ops on the default
    device) so the floor/log10 bin boundaries match bit-for-bit."""
    import jax.numpy as jnp

    cv = jnp.asarray(np.asarray(coord_v, dtype=np.float32))
    n = cv.shape[1] // 3
    v10 = cv.at[:, 2::3].set(jnp.log10(cv[:, 2::3]))
    lo = jnp.tile(jnp.asarray(np.asarray(lows, dtype=np.float32)), n)
    hi = jnp.tile(jnp.asarray(np.asarray(highs, dtype=np.float32)), n)
    coord_grid = (v10 - lo) / (hi - lo)
    tr = coord_grid.reshape(-1, 3)
    x_i = jnp.floor(tr[:, 0] * L_).astype(jnp.int32)
    y_i = jnp.floor(tr[:, 1] * L_).astype(jnp.int32)
    m_i = jnp.floor(tr[:, 2] * nmc).astype(jnp.int32)
    return (np.asarray(x_i), np.asarray(y_i), np.asarray(m_i))


def _prepare_in_maps(coord_v, lows, highs, nmc, L):
    nmc = int(nmc)
    L_ = int(L)
    x_i, y_i, m_i = _compute_indices(coord_v, lows, highs, nmc, L_)
    n_batch = coord_v.shape[0]
    n = coord_v.shape[1] // 3
    b_i = np.repeat(np.arange(n_batch, dtype=np.int64), n)

    # Flat element offsets (per core, local slab coordinates).
    flat_ones = ((b_i % BL) * SLAB + m_i.astype(np.int64) * PLANE
                 + y_i.astype(np.int64) * L_ + x_i.astype(np.int64))
    flat_z = flat_ones + HALF

    in_maps = []
    for c in range(NCORES):
        sel = slice(c * PTS, (c + 1) * PTS)
        po = flat_ones[sel]
        pz = flat_z[sel]
        offs_np = np.zeros((128, N_SCATTER_COLS), dtype=np.int32)
        for j in range(N_SCATTER_COLS):
            offs_np[0:64, j] = po[64 * j:64 * (j + 1)]
            offs_np[64:128, j] = pz[64 * j:64 * (j + 1)]
        in_maps.append({"offs": offs_np})
    return in_maps


def _run(in_maps, **kwargs):
    if "nc" not in _CACHE:
        _CACHE["nc"] = _build_nc()
    nc = _CACHE["nc"]
    from concourse.bass_utils import run_bass_kernel_spmd
    return run_bass_kernel_spmd(nc, in_maps, core_ids=list(range(NCORES)),
                                **kwargs)


def kernel(coord_v, lows, highs, nmc, L):
    nmc = int(nmc)
    L_ = int(L)
    assert nmc == NMC and L_ == globals()["L"], (nmc, L_)

    in_maps = _prepare_in_maps(coord_v, lows, highs, nmc, L_)
    res = _run(in_maps)
    parts = [res.results[c]["out"].reshape(BL, 2 * NMC, L_, L_)
             for c in range(NCORES)]
    return np.concatenate(parts, axis=0)


# revision 26
# speedup vs baseline: 1.1147x; 1.1147x over previous
"""Trainium2 Bass kernel for nn_CustomParameterTransform (scatter_memory).

Reference semantics: coord_v [256, 30] holds 10 (x, y, mass) triplets per
sample. Each triplet maps to integer grid indices (x_i, y_i, m_i); a one-hot
volume z [B, 16, 128, 128] is scattered (z[b, m, y, x] = 1) and the output is
concat(1-z, z) over the channel axis -> [256, 32, 128, 128] f32 (512 MB).

Strategy (8 NeuronCores, batch-sharded, no cross-core comm):
  - The output is almost entirely constant (first 16 channels 1.0, last 16
    0.0, except at 640 scatter points per core). Per core: one 64 MB
    write-only region built from SBUF "slab image" tiles whose
    partition-major sweep reproduces whole slabs (alternating 1 MB ones /
    1 MB zeros), so every fill is a contiguous DRAM write and both DMA
    sides stay 2-D (the HWDGE PDMA2D fast path; 3-D/strided APs demote to
    an engine-sequenced slow path measured ~5x slower).
  - Steady-state throughput is capped by the per-core DMA port (~435
    GB/s); a ring dispatches ~4 descriptors/us per outstanding
    instruction, so the ramp is limited by how quickly fill instructions
    become ready. Hence: a [128, 1024] mini tile (one ~0.9 us memset per
    engine) feeds the very first fills, slabs 1-5 are ten 1 MB half-slab
    fills (lots of outstanding instructions early), and slabs 6+ are 4 MB
    fills from a [128, 8192] tile (32 KB rows; rows can't exceed 32 KB
    because a larger slab image needs its value to alternate every <32
    partitions and compute APs must start on 32-partition quadrant
    boundaries).
  - A gpsimd software-DGE fill queue was tried as a third descriptor
    stream and made things worse: engines stall fetching software
    descriptors, throttling the HWDGE rings. Everything stays on the two
    rings.
  - The 640 scatter points are fixed up with indirect (scatter) DMAs whose
    deps are wired to just the fills covering their samples, so all but
    the last column overlap the fill phase.
  - The stock const-AP all-engine barrier in Bass.__init__ is patched out
    (nothing here uses const_aps) and TileContext's epilogue is replaced
    with a light drain, since the event-lowered sem-clear cascade scales
    with instruction count.
  - Indices are computed on the host with the exact same jax ops as the
    reference (bit-identical floor/log10 behavior) and passed per-core as
    a [128, 5] int32 tensor of flat element offsets.
"""

import numpy as np

B = 256
NSRC = 10
NMC = 16
L = 128
NCORES = 8
BL = B // NCORES          # 32 samples per core
PLANE = L * L             # 16384
HALF = NMC * PLANE        # 262144 elements per half-slab
SLAB = 2 * HALF           # 524288 elements per sample
OUT_ELEMS = BL * SLAB     # 16777216 per core (64 MB)

N_SCATTER_COLS = 5        # 640 scatter writes = 128 partitions x 5 columns
PTS = BL * NSRC           # 320 points per core

_CACHE = {}


def _build_nc():
    import concourse.bass as bass
    import concourse.tile as tile
    from concourse import bacc, mybir
    from concourse.tile_rust import add_dep_helper

    import types as _types
    from concourse.vector_clock import ScopedClock

    # The const-AP registration in Bass.__init__ ends with an all-engine
    # barrier (~1.5 us of event-sem chaining at the head of every
    # execution). This kernel never touches const_aps -- memset packs its
    # immediate and the DMAs don't use them -- so elide the barrier for
    # the duration of construction.
    _orig_barrier = bass.Bass.all_engine_barrier
    bass.Bass.all_engine_barrier = lambda self, **kw: None
    try:
        nc = bacc.Bacc("TRN2", target_bir_lowering=False, debug=False,
                       num_devices=NCORES)
    finally:
        bass.Bass.all_engine_barrier = _orig_barrier

    def _light_drain_and_barrier(self, tick_clock, wait_clock):
        """Replaces TileContext._drain_and_barrier for this kernel. The
        stock epilogue is drain + two all-engine EVSEM butterfly barriers
        around the sem clear (~9 us after event lowering). Requirements at
        kernel end are: (1) all DMA completions observed, (2) sems cleared
        for NEFF re-execution, (3) the clear happens after every engine's
        last sem use. (1) is the sync drain's global-clock waits; (3) is a
        counting-sem join (sync arrives only after the drain, so join>=4
        implies all DMA done); (2) is the ranged clear. The second barrier
        is unnecessary: a re-execution cannot start until every engine --
        including the clearing gpsimd -- has ended."""
        nc_ = self.nc
        drain_inst = nc_.sync.drain()
        wait_clock.add_sem_waits(
            drain_inst.ins, ScopedClock({None: tick_clock.global_clock}))
        join = nc_.alloc_semaphore("tail_join")
        for eng in nc_.engines.values():
            if eng is not nc_.gpsimd:
                eng.sem_inc(join, 1)
        n_other = len(nc_.engines) - 1
        nc_.gpsimd.wait_ge(join, n_other)
        popped = nc_._tile_sem_poison_stack.pop()
        assert popped is self._sem_poison
        sems = list(self.sems.allocated().values())
        nc_.clear_and_free_semaphores(sems + [join])

    offs = nc.dram_tensor("offs", [128, N_SCATTER_COLS], mybir.dt.int32,
                          kind="ExternalInput").ap()
    out = nc.dram_tensor("out", [BL, SLAB], mybir.dt.float32,
                         kind="ExternalOutput").ap()

    with tile.TileContext(nc) as tc:
        tc._drain_and_barrier = _types.MethodType(_light_drain_and_barrier, tc)
        with tc.tile_pool(name="src", bufs=1) as src_pool, \
             tc.tile_pool(name="small", bufs=1) as small_pool:
            # Mini tiles: first memset on each of vector/gpsimd (~0.9 us)
            # so the rings' first fills push as early as possible.
            ones_mini = src_pool.tile([128, 1024], mybir.dt.float32)
            zeros_mini = src_pool.tile([128, 1024], mybir.dt.float32)
            nc.vector.memset(ones_mini[:, :], 1.0)
            nc.gpsimd.memset(zeros_mini[:, :], 0.0)
            # Stage A [128, 4096] (1 slab/sweep, 16 KB rows): ones rows
            # ready next (~2 us later), zeros rows after that.
            slab_a = src_pool.tile([128, 4096], mybir.dt.float32)
            nc.vector.memset(slab_a[0:64, 0:2048], 1.0)
            nc.gpsimd.memset(slab_a[0:64, 2048:4096], 1.0)
            nc.vector.memset(slab_a[64:128, 0:2048], 0.0)
            nc.gpsimd.memset(slab_a[64:128, 2048:4096], 0.0)
            # Stage B [128, 8192] (2 slabs/sweep, 32 KB rows, value
            # alternating every 32 rows), columns split vector/gpsimd
            # (scalar and sync cannot memset).
            slab_b = src_pool.tile([128, 8192], mybir.dt.float32)
            for r in range(4):
                v = 1.0 if r % 2 == 0 else 0.0
                nc.vector.memset(slab_b[r * 32:(r + 1) * 32, 0:4096], v)
                nc.gpsimd.memset(slab_b[r * 32:(r + 1) * 32, 4096:8192], v)

            # Scatter offsets: [128, 5] int32 flat element indices.
            # Column j: rows 0-63 = ones-half offsets of points
            # 64j..64j+63 (write 0.0), rows 64-127 = z-half offsets of the
            # same points (write 1.0) -- vals_t is just two quadrant-
            # aligned memsets. These queue behind the gpsimd memsets; the
            # scatters need them ~50 us in.
            offs_t = small_pool.tile([128, N_SCATTER_COLS], mybir.dt.int32)
            nc.gpsimd.dma_start(offs_t[:, :], offs[:, :])
            vals_t = small_pool.tile([128, N_SCATTER_COLS], mybir.dt.float32)
            nc.gpsimd.memset(vals_t[0:64, :], 0.0)
            nc.gpsimd.memset(vals_t[64:128, :], 1.0)

            # Fills. sample_fills[s] lists the fills that write slab s.
            #   slab 0:    4 half-MB fills from the minis (earliest start)
            #   slabs 1-5: ten 1 MB half-slab fills from stage A -- many
            #              small instructions so the rings have descriptor
            #              sources queued while stage B memsets finish
            #   slabs 6-27: eleven 4 MB stage-B fills on the rings
            #   slabs 28-31: two 4 MB stage-B fills on gpsimd's software
            #              DGE queue (third descriptor stream)
            sample_fills = {s: [] for s in range(BL)}
            for k in range(2):
                f = nc.sync.dma_start(
                    out[0:1, k * HALF // 2:(k + 1) * HALF // 2],
                    ones_mini[:, :])
                sample_fills[0].append(f)
                f = nc.scalar.dma_start(
                    out[0:1, HALF + k * HALF // 2:HALF + (k + 1) * HALF // 2],
                    zeros_mini[:, :])
                sample_fills[0].append(f)
            for s in range(1, 6):
                f = nc.sync.dma_start(out[s:s + 1, 0:HALF], slab_a[0:64, :])
                sample_fills[s].append(f)
                f = nc.scalar.dma_start(out[s:s + 1, HALF:SLAB],
                                        slab_a[64:128, :])
                sample_fills[s].append(f)
            # Slabs 6-13: 2 MB single-slab fills (more outstanding
            # instructions right as stage B comes ready -- the ring
            # dispatches ~4 descriptors/us per outstanding instruction).
            # Slabs 14-29: 4 MB fills. Slabs 30-31 split 1-each so both
            # rings carry exactly 32 MB (an imbalanced ring shows up as
            # ~7 us of late engines on the heavy ring).
            # Descriptors are served by the DMA engine owning the source
            # partition group (partition//8), so a fill reading only rows
            # 0-63 runs on half the engines; rows 64-127 of slab_b are an
            # identical slab image, and the two rings read opposite halves
            # to cover all 16 engines.
            for i, s in enumerate(range(6, 14)):
                eng = nc.sync if i % 2 == 0 else nc.scalar
                src = slab_b[0:64, :] if i % 2 == 0 else slab_b[64:128, :]
                f = eng.dma_start(out[s:s + 1, :], src)
                sample_fills[s].append(f)
            for i, s in enumerate(range(14, 30, 2)):
                eng = nc.sync if i % 2 == 0 else nc.scalar
                f = eng.dma_start(out[s:s + 2, :].flatten(), slab_b[:, :])
                for ss in (s, s + 1):
                    sample_fills[ss].append(f)
            sample_fills[30].append(
                nc.sync.dma_start(out[30:31, :], slab_b[0:64, :]))
            sample_fills[31].append(
                nc.scalar.dma_start(out[31:32, :], slab_b[64:128, :]))

            # Scatter columns: col j covers points 64j..64j+63, i.e.
            # samples floor(6.4j)..floor(6.4j+6.4).
            def deps(lo, hi):
                seen = []
                for s in range(lo, hi + 1):
                    for f in sample_fills[s]:
                        if f not in seen:
                            seen.append(f)
                return seen
            col_deps = [deps(0, 6), deps(6, 12), deps(12, 19),
                        deps(19, 25), deps(25, 31)]

            # Narrow declared out AP ([1, 1] at offset 0): the real write
            # addresses come from the offset tensor; a full-tensor AP would
            # make Tile serialize every scatter behind every fill (WAW), and
            # the explicit col_deps edges below provide the true ordering.
            out2d = out[0:1, 0:1]
            for j in range(N_SCATTER_COLS):
                sc = nc.gpsimd.indirect_dma_start(
                    out=out2d,
                    out_offset=bass.IndirectOffsetOnAxis(
                        ap=offs_t[:, j:j + 1], axis=0),
                    in_=vals_t[:, j:j + 1],
                    in_offset=None,
                )
                for fl in col_deps[j]:
                    add_dep_helper(sc.ins, fl.ins,
                                   reason="scatter after its sample fills")

    nc.compile()
    return nc


def _compute_indices(coord_v, lows, highs, nmc, L_):
    """Replicates reference.py lines exactly (same jax ops on the default
    device) so the floor/log10 bin boundaries match bit-for-bit."""
    import jax.numpy as jnp

    cv = jnp.asarray(np.asarray(coord_v, dtype=np.float32))
    n = cv.shape[1] // 3
    v10 = cv.at[:, 2::3].set(jnp.log10(cv[:, 2::3]))
    lo = jnp.tile(jnp.asarray(np.asarray(lows, dtype=np.float32)), n)
    hi = jnp.tile(jnp.asarray(np.asarray(highs, dtype=np.float32)), n)
    coord_grid = (v10 - lo) / (hi - lo)
    tr = coord_grid.reshape(-1, 3)
    x_i = jnp.floor(tr[:, 0] * L_).astype(jnp.int32)
    y_i = jnp.floor(tr[:, 1] * L_).astype(jnp.int32)
    m_i = jnp.floor(tr[:, 2] * nmc).astype(jnp.int32)
    return (np.asarray(x_i), np.asarray(y_i), np.asarray(m_i))


def _prepare_in_maps(coord_v, lows, highs, nmc, L):
    nmc = int(nmc)
    L_ = int(L)
    x_i, y_i, m_i = _compute_indices(coord_v, lows, highs, nmc, L_)
    n_batch = coord_v.shape[0]
    n = coord_v.shape[1] // 3
    b_i = np.repeat(np.arange(n_batch, dtype=np.int64), n)

    # Flat element offsets (per core, local slab coordinates).
    flat_ones = ((b_i % BL) * SLAB + m_i.astype(np.int64) * PLANE
                 + y_i.astype(np.int64) * L_ + x_i.astype(np.int64))
    flat_z = flat_ones + HALF

    in_maps = []
    for c in range(NCORES):
        sel = slice(c * PTS, (c + 1) * PTS)
        po = flat_ones[sel]
        pz = flat_z[sel]
        offs_np = np.zeros((128, N_SCATTER_COLS), dtype=np.int32)
        for j in range(N_SCATTER_COLS):
            offs_np[0:64, j] = po[64 * j:64 * (j + 1)]
            offs_np[64:128, j] = pz[64 * j:64 * (j + 1)]
        in_maps.append({"offs": offs_np})
    return in_maps


def _run(in_maps, **kwargs):
    if "nc" not in _CACHE:
        _CACHE["nc"] = _build_nc()
    nc = _CACHE["nc"]
    from concourse.bass_utils import run_bass_kernel_spmd
    return run_bass_kernel_spmd(nc, in_maps, core_ids=list(range(NCORES)),
                                **kwargs)


def kernel(coord_v, lows, highs, nmc, L):
    nmc = int(nmc)
    L_ = int(L)
    assert nmc == NMC and L_ == globals()["L"], (nmc, L_)

    in_maps = _prepare_in_maps(coord_v, lows, highs, nmc, L_)
    res = _run(in_maps)
    parts = [res.results[c]["out"].reshape(BL, 2 * NMC, L_, L_)
             for c in range(NCORES)]
    return np.concatenate(parts, axis=0)


# revision 27
# speedup vs baseline: 1.2804x; 1.1487x over previous
"""Trainium2 Bass kernel for nn_CustomParameterTransform (scatter_memory).

Reference semantics: coord_v [256, 30] holds 10 (x, y, mass) triplets per
sample. Each triplet maps to integer grid indices (x_i, y_i, m_i); a one-hot
volume z [B, 16, 128, 128] is scattered (z[b, m, y, x] = 1) and the output is
concat(1-z, z) over the channel axis -> [256, 32, 128, 128] f32 (512 MB).

Strategy (8 NeuronCores, batch-sharded, no cross-core comm):
  - The output is almost entirely constant (first 16 channels 1.0, last 16
    0.0, except at 640 scatter points per core). Per core: one 64 MB
    write-only region built from SBUF "slab image" tiles whose
    partition-major sweep reproduces whole slabs (alternating 1 MB ones /
    1 MB zeros), so every fill is a contiguous DRAM write and both DMA
    sides stay 2-D (the HWDGE PDMA2D fast path; 3-D/strided APs demote to
    an engine-sequenced slow path measured ~5x slower).
  - Steady-state throughput is capped by the per-core DMA port (~435
    GB/s); a ring dispatches ~4 descriptors/us per outstanding
    instruction, so the ramp is limited by how quickly fill instructions
    become ready. Hence: a [128, 1024] mini tile (one ~0.9 us memset per
    engine) feeds the very first fills, slabs 1-5 are ten 1 MB half-slab
    fills (lots of outstanding instructions early), and slabs 6+ are 4 MB
    fills from a [128, 8192] tile (32 KB rows; rows can't exceed 32 KB
    because a larger slab image needs its value to alternate every <32
    partitions and compute APs must start on 32-partition quadrant
    boundaries).
  - A gpsimd software-DGE fill queue was tried as a third descriptor
    stream and made things worse: engines stall fetching software
    descriptors, throttling the HWDGE rings. Everything stays on the two
    rings.
  - The 640 scatter points are fixed up with indirect (scatter) DMAs whose
    deps are wired to just the fills covering their samples, so all but
    the last column overlap the fill phase.
  - The stock const-AP all-engine barrier in Bass.__init__ is patched out
    (nothing here uses const_aps) and TileContext's epilogue is replaced
    with a light drain, since the event-lowered sem-clear cascade scales
    with instruction count.
  - Indices are computed on the host with the exact same jax ops as the
    reference (bit-identical floor/log10 behavior) and passed per-core as
    a [128, 5] int32 tensor of flat element offsets.
"""

import numpy as np

B = 256
NSRC = 10
NMC = 16
L = 128
NCORES = 8
BL = B // NCORES          # 32 samples per core
PLANE = L * L             # 16384
HALF = NMC * PLANE        # 262144 elements per half-slab
SLAB = 2 * HALF           # 524288 elements per sample
OUT_ELEMS = BL * SLAB     # 16777216 per core (64 MB)

N_SCATTER_COLS = 5        # 640 scatter writes = 128 partitions x 5 columns
PTS = BL * NSRC           # 320 points per core

_CACHE = {}


def _build_nc():
    import concourse.bass as bass
    import concourse.tile as tile
    from concourse import bacc, mybir
    from concourse.tile_rust import add_dep_helper

    import types as _types
    from concourse.vector_clock import ScopedClock

    # The const-AP registration in Bass.__init__ ends with an all-engine
    # barrier (~1.5 us of event-sem chaining at the head of every
    # execution). This kernel never touches const_aps -- memset packs its
    # immediate and the DMAs don't use them -- so elide the barrier for
    # the duration of construction.
    _orig_barrier = bass.Bass.all_engine_barrier
    bass.Bass.all_engine_barrier = lambda self, **kw: None
    try:
        nc = bacc.Bacc("TRN2", target_bir_lowering=False, debug=False,
                       num_devices=NCORES)
    finally:
        bass.Bass.all_engine_barrier = _orig_barrier

    def _light_drain_and_barrier(self, tick_clock, wait_clock):
        """Replaces TileContext._drain_and_barrier for this kernel. The
        stock epilogue is drain + two all-engine EVSEM butterfly barriers
        around the sem clear (~9 us after event lowering). Requirements at
        kernel end are: (1) all DMA completions observed, (2) sems cleared
        for NEFF re-execution, (3) the clear happens after every engine's
        last sem use. (1) is the sync drain's global-clock waits; (3) is a
        counting-sem join (sync arrives only after the drain, so join>=4
        implies all DMA done); (2) is the ranged clear. The second barrier
        is unnecessary: a re-execution cannot start until every engine --
        including the clearing gpsimd -- has ended."""
        nc_ = self.nc
        drain_inst = nc_.sync.drain()
        wait_clock.add_sem_waits(
            drain_inst.ins, ScopedClock({None: tick_clock.global_clock}))
        join = nc_.alloc_semaphore("tail_join")
        for eng in nc_.engines.values():
            if eng is not nc_.gpsimd:
                eng.sem_inc(join, 1)
        n_other = len(nc_.engines) - 1
        nc_.gpsimd.wait_ge(join, n_other)
        popped = nc_._tile_sem_poison_stack.pop()
        assert popped is self._sem_poison
        sems = list(self.sems.allocated().values())
        nc_.clear_and_free_semaphores(sems + [join])

    offs = nc.dram_tensor("offs", [128, N_SCATTER_COLS], mybir.dt.int32,
                          kind="ExternalInput").ap()
    out = nc.dram_tensor("out", [BL, SLAB], mybir.dt.float32,
                         kind="ExternalOutput").ap()

    with tile.TileContext(nc) as tc:
        tc._drain_and_barrier = _types.MethodType(_light_drain_and_barrier, tc)
        with tc.tile_pool(name="src", bufs=1) as src_pool, \
             tc.tile_pool(name="small", bufs=1) as small_pool:
            # Mini tiles: first memset on each of vector/gpsimd (~0.9 us)
            # so the rings' first fills push as early as possible.
            ones_mini = src_pool.tile([128, 1024], mybir.dt.float32)
            zeros_mini = src_pool.tile([128, 1024], mybir.dt.float32)
            nc.vector.memset(ones_mini[:, :], 1.0)
            nc.gpsimd.memset(zeros_mini[:, :], 0.0)
            # Stage A [128, 4096] (1 slab/sweep, 16 KB rows): ones rows
            # ready next (~2 us later), zeros rows after that.
            slab_a = src_pool.tile([128, 4096], mybir.dt.float32)
            nc.vector.memset(slab_a[0:64, 0:2048], 1.0)
            nc.gpsimd.memset(slab_a[0:64, 2048:4096], 1.0)
            nc.vector.memset(slab_a[64:128, 0:2048], 0.0)
            nc.gpsimd.memset(slab_a[64:128, 2048:4096], 0.0)
            # Stage B [128, 8192] (2 slabs/sweep, 32 KB rows, value
            # alternating every 32 rows), columns split vector/gpsimd
            # (scalar and sync cannot memset).
            slab_b = src_pool.tile([128, 8192], mybir.dt.float32)
            for r in range(4):
                v = 1.0 if r % 2 == 0 else 0.0
                nc.vector.memset(slab_b[r * 32:(r + 1) * 32, 0:4096], v)
                nc.gpsimd.memset(slab_b[r * 32:(r + 1) * 32, 4096:8192], v)

            # Scatter offsets: [128, 5] int32 flat element indices.
            # Column j: rows 0-63 = ones-half offsets of points
            # 64j..64j+63 (write 0.0), rows 64-127 = z-half offsets of the
            # same points (write 1.0) -- vals_t is just two quadrant-
            # aligned memsets. These queue behind the gpsimd memsets; the
            # scatters need them ~50 us in.
            offs_t = small_pool.tile([128, N_SCATTER_COLS], mybir.dt.int32)
            nc.gpsimd.dma_start(offs_t[:, :], offs[:, :])
            vals_t = small_pool.tile([128, N_SCATTER_COLS], mybir.dt.float32)
            nc.gpsimd.memset(vals_t[0:64, :], 0.0)
            nc.gpsimd.memset(vals_t[64:128, :], 1.0)

            # Fills. sample_fills[s] lists the fills that write slab s.
            #   slab 0:    4 half-MB fills from the minis (earliest start)
            #   slabs 1-5: ten 1 MB half-slab fills from stage A -- many
            #              small instructions so the rings have descriptor
            #              sources queued while stage B memsets finish
            #   slabs 6-27: eleven 4 MB stage-B fills on the rings
            #   slabs 28-31: two 4 MB stage-B fills on gpsimd's software
            #              DGE queue (third descriptor stream)
            sample_fills = {s: [] for s in range(BL)}
            for k in range(2):
                f = nc.sync.dma_start(
                    out[0:1, k * HALF // 2:(k + 1) * HALF // 2],
                    ones_mini[:, :])
                sample_fills[0].append(f)
                f = nc.scalar.dma_start(
                    out[0:1, HALF + k * HALF // 2:HALF + (k + 1) * HALF // 2],
                    zeros_mini[:, :])
                sample_fills[0].append(f)
            for s in range(1, 6):
                f = nc.sync.dma_start(out[s:s + 1, 0:HALF], slab_a[0:64, :])
                sample_fills[s].append(f)
                f = nc.scalar.dma_start(out[s:s + 1, HALF:SLAB],
                                        slab_a[64:128, :])
                sample_fills[s].append(f)
            # Slabs 6-29: 4 MB whole-tile fills -- descriptors are served
            # by the DMA engine owning the source partition group
            # (partition//8), so only fills reading all 128 rows engage
            # all 16 engines on their own (half-row sources measured at
            # exactly half rate when both rings read the same half).
            # Slabs 30-31: one single each from OPPOSITE tile halves (rows
            # 64-127 are an identical slab image) so both rings carry
            # exactly 32 MB and the tail keeps full engine coverage.
            for i, s in enumerate(range(6, 30, 2)):
                eng = nc.sync if i % 2 == 0 else nc.scalar
                f = eng.dma_start(out[s:s + 2, :].flatten(), slab_b[:, :])
                for ss in (s, s + 1):
                    sample_fills[ss].append(f)
            sample_fills[30].append(
                nc.sync.dma_start(out[30:31, :], slab_b[0:64, :]))
            sample_fills[31].append(
                nc.scalar.dma_start(out[31:32, :], slab_b[64:128, :]))

            # Scatter columns: col j covers points 64j..64j+63, i.e.
            # samples floor(6.4j)..floor(6.4j+6.4).
            def deps(lo, hi):
                seen = []
                for s in range(lo, hi + 1):
                    for f in sample_fills[s]:
                        if f not in seen:
                            seen.append(f)
                return seen
            col_deps = [deps(0, 6), deps(6, 12), deps(12, 19),
                        deps(19, 25), deps(25, 31)]

            # Narrow declared out AP ([1, 1] at offset 0): the real write
            # addresses come from the offset tensor; a full-tensor AP would
            # make Tile serialize every scatter behind every fill (WAW), and
            # the explicit col_deps edges below provide the true ordering.
            out2d = out[0:1, 0:1]
            for j in range(N_SCATTER_COLS):
                sc = nc.gpsimd.indirect_dma_start(
                    out=out2d,
                    out_offset=bass.IndirectOffsetOnAxis(
                        ap=offs_t[:, j:j + 1], axis=0),
                    in_=vals_t[:, j:j + 1],
                    in_offset=None,
                )
                for fl in col_deps[j]:
                    add_dep_helper(sc.ins, fl.ins,
                                   reason="scatter after its sample fills")

    nc.compile()
    return nc


def _compute_indices(coord_v, lows, highs, nmc, L_):
    """Replicates reference.py lines exactly (same jax ops on the default
    device) so the floor/log10 bin boundaries match bit-for-bit."""
    import jax.numpy as jnp

    cv = jnp.asarray(np.asarray(coord_v, dtype=np.float32))
    n = cv.shape[1] // 3
    v10 = cv.at[:, 2::3].set(jnp.log10(cv[:, 2::3]))
    lo = jnp.tile(jnp.asarray(np.asarray(lows, dtype=np.float32)), n)
    hi = jnp.tile(jnp.asarray(np.asarray(highs, dtype=np.float32)), n)
    coord_grid = (v10 - lo) / (hi - lo)
    tr = coord_grid.reshape(-1, 3)
    x_i = jnp.floor(tr[:, 0] * L_).astype(jnp.int32)
    y_i = jnp.floor(tr[:, 1] * L_).astype(jnp.int32)
    m_i = jnp.floor(tr[:, 2] * nmc).astype(jnp.int32)
    return (np.asarray(x_i), np.asarray(y_i), np.asarray(m_i))


def _prepare_in_maps(coord_v, lows, highs, nmc, L):
    nmc = int(nmc)
    L_ = int(L)
    x_i, y_i, m_i = _compute_indices(coord_v, lows, highs, nmc, L_)
    n_batch = coord_v.shape[0]
    n = coord_v.shape[1] // 3
    b_i = np.repeat(np.arange(n_batch, dtype=np.int64), n)

    # Flat element offsets (per core, local slab coordinates).
    flat_ones = ((b_i % BL) * SLAB + m_i.astype(np.int64) * PLANE
                 + y_i.astype(np.int64) * L_ + x_i.astype(np.int64))
    flat_z = flat_ones + HALF

    in_maps = []
    for c in range(NCORES):
        sel = slice(c * PTS, (c + 1) * PTS)
        po = flat_ones[sel]
        pz = flat_z[sel]
        offs_np = np.zeros((128, N_SCATTER_COLS), dtype=np.int32)
        for j in range(N_SCATTER_COLS):
            offs_np[0:64, j] = po[64 * j:64 * (j + 1)]
            offs_np[64:128, j] = pz[64 * j:64 * (j + 1)]
        in_maps.append({"offs": offs_np})
    return in_maps


def _run(in_maps, **kwargs):
    if "nc" not in _CACHE:
        _CACHE["nc"] = _build_nc()
    nc = _CACHE["nc"]
    from concourse.bass_utils import run_bass_kernel_spmd
    return run_bass_kernel_spmd(nc, in_maps, core_ids=list(range(NCORES)),
                                **kwargs)


def kernel(coord_v, lows, highs, nmc, L):
    nmc = int(nmc)
    L_ = int(L)
    assert nmc == NMC and L_ == globals()["L"], (nmc, L_)

    in_maps = _prepare_in_maps(coord_v, lows, highs, nmc, L_)
    res = _run(in_maps)
    parts = [res.results[c]["out"].reshape(BL, 2 * NMC, L_, L_)
             for c in range(NCORES)]
    return np.concatenate(parts, axis=0)
